# revision 1
# baseline (speedup 1.0000x reference)
"""Multi-head attention (B=4, S=2048, D=1024, H=16) on 8 Trainium2 cores.

Sharding: core c -> (batch b=c//2, query-half hq=c%2). Each core computes
K/V projections for its batch's full sequence (no collectives needed) and
attention + output projection for its 1024 query rows.

Device dataflow (all activations kept transposed, [feature, seq]):
  qT[e,s]   = WqT.T-contract  (lhsT=WqT[d,e] tiles, rhs=xT[d,s])
  kT[e,s]   = same with WkT
  v[s,e]    = lhsT=xT[d,s] tiles, rhs=WvT[d,e]  (+bias via K=1 ones matmul)
  per head, per 512-q chunk:
    scoresT[k,q] = kT_h.T-contract q  (K=64 matmuls, 4 k-tiles -> 4 psum banks)
    expT = ScalarE Exp(scale=0.125) over [128, 2048] psum -> bf16 sbuf
    ctxT[dv,q]  += [v_h | ones] @ expT   (row 64 = softmax denominator)
    normalize: reciprocal + PE broadcast outer-product + DVE multiply
  outT[e,q] = WoT.T-contract ctxnT  (bias bo added host-side)
Host: out[b, hq*1024:(hq+1)*1024, :] = outT.T + bo
"""

import numpy as np
import ml_dtypes

import concourse.bacc as bacc
import concourse.tile as tile
from concourse import mybir
from concourse.bass_utils import run_bass_kernel_spmd

B, S, D = 4, 2048, 1024
H, HD = 16, 64
SQ = 1024          # query rows per core
NDT = D // 128     # 8 d-tiles
NET = D // 128     # 8 e-tiles
NKT = S // 128     # 16 k-tiles
NST = S // 128     # 16 s-tiles
NQC = SQ // 512    # 2 q-chunks per core
BF16 = mybir.dt.bfloat16
F32 = mybir.dt.float32
SCALE = 1.0 / 8.0  # 1/sqrt(HD)

_NC_CACHE = None


def build_nc():
    nc = bacc.Bacc(None, target_bir_lowering=False, debug=True)

    xT_d = nc.declare_dram_parameter("xT", [D, S], BF16, isOutput=False)
    WqT_d = nc.declare_dram_parameter("WqT", [D, D], BF16, isOutput=False)
    WkT_d = nc.declare_dram_parameter("WkT", [D, D], BF16, isOutput=False)
    WvT_d = nc.declare_dram_parameter("WvT", [D, D], BF16, isOutput=False)
    WoT_d = nc.declare_dram_parameter("WoT", [D, D], BF16, isOutput=False)
    bqt_d = nc.declare_dram_parameter("bqt", [128, NET], F32, isOutput=False)
    bkt_d = nc.declare_dram_parameter("bkt", [128, NET], F32, isOutput=False)
    bvr_d = nc.declare_dram_parameter("bvr", [1, D], BF16, isOutput=False)
    outT_d = nc.declare_dram_parameter("outT", [D, SQ], F32, isOutput=True)

    with tile.TileContext(nc) as tc:
        with tc.tile_pool(name="resident", bufs=1) as res:
            # ---- resident SBUF tensors ----
            kT = [res.tile([128, S], BF16, name=f"kT{t}", tag=f"kT{t}")
                  for t in range(NET)]
            qT = [res.tile([128, SQ], BF16, name=f"qT{t}", tag=f"qT{t}")
                  for t in range(NET)]
            vv = [res.tile([128, H, HD + 1], BF16, name=f"v{t}", tag=f"v{t}")
                  for t in range(NST)]
            ctxn = [res.tile([128, SQ], BF16, name=f"ctxn{t}", tag=f"ctxn{t}")
                    for t in range(NDT)]
            Wo_t = [res.tile([128, D], BF16, name=f"Wo{t}", tag=f"Wo{t}")
                    for t in range(NDT)]
            bq_dma = res.tile([128, NET], F32, tag="bq_dma")
            bk_dma = res.tile([128, NET], F32, tag="bk_dma")
            bq_sb = res.tile([128, NET], F32, tag="bq_sb")
            bk_sb = res.tile([128, NET], F32, tag="bk_sb")
            bv_sb = res.tile([1, D], BF16, tag="bv_sb")
            ones_bf = res.tile([1, 128], BF16, tag="ones_bf")
            ones_r = res.tile([65, 64], F32, tag="ones_r")

            nc.sync.dma_start(out=bq_dma, in_=bqt_d[:, :])
            nc.sync.dma_start(out=bk_dma, in_=bkt_d[:, :])
            nc.sync.dma_start(out=bv_sb, in_=bvr_d[:, :])
            # TensorScalarPtr has a single sync-wait slot; route the biases
            # through DVE once so later readers rely on program order.
            nc.vector.tensor_copy(out=bq_sb, in_=bq_dma)
            nc.vector.tensor_copy(out=bk_sb, in_=bk_dma)
            nc.vector.memset(ones_bf, 1.0)
            nc.vector.memset(ones_r, 1.0)
            for t in range(NST):
                # only the denominator column; cols 0:HD are overwritten
                nc.vector.memset(vv[t][:, :, HD:HD + 1], 1.0)

            # ================= phase 1: projections =================
            with tc.tile_pool(name="p1", bufs=1) as p1, \
                 tc.psum_pool(name="pp", bufs=4) as pp:
                xT = [p1.tile([128, S], BF16, name=f"xT{t}", tag=f"xT{t}")
                      for t in range(NDT)]
                for t in range(NDT):
                    nc.sync.dma_start(out=xT[t], in_=xT_d[t * 128:(t + 1) * 128, :])

                # qT then kT: out[e_tile, s_chunk] accumulated over d
                for W_d, out_tiles, bias_sb, nsc in (
                    (WqT_d, qT, bq_sb, NQC),   # q: only local 1024 cols
                    (WkT_d, kT, bk_sb, S // 512),
                ):
                    w_t = []
                    for t in range(NDT):
                        wt = p1.tile([128, D], BF16, name=f"w{t}", tag="wrot",
                                     bufs=10)
                        nc.sync.dma_start(out=wt, in_=W_d[t * 128:(t + 1) * 128, :])
                        w_t.append(wt)
                    for et in range(NET):
                        for sc in range(nsc):
                            ps = pp.tile([128, 512], F32, name="ps", tag="proj")
                            for dt in range(NDT):
                                nc.tensor.matmul(
                                    ps,
                                    w_t[dt][:, et * 128:(et + 1) * 128],
                                    xT[dt][:, sc * 512: sc * 512 + 512],
                                    start=(dt == 0), stop=(dt == NDT - 1))
                            nc.vector.tensor_scalar_add(
                                out=out_tiles[et][:, sc * 512:(sc + 1) * 512],
                                in0=ps,
                                scalar1=bias_sb[:, et:et + 1])

                # v: out[s_tile, e_chunk] accumulated over d, + ones-row bias
                wv_t = []
                for t in range(NDT):
                    wt = p1.tile([128, D], BF16, name=f"wv{t}", tag="wrot",
                                 bufs=10)
                    nc.sync.dma_start(out=wt, in_=WvT_d[t * 128:(t + 1) * 128, :])
                    wv_t.append(wt)
                for st in range(NST):
                    for ec in range(D // 512):
                        ps = pp.tile([128, 512], F32, name="ps", tag="proj")
                        for dt in range(NDT):
                            nc.tensor.matmul(
                                ps,
                                xT[dt][:, st * 128:(st + 1) * 128],
                                wv_t[dt][:, ec * 512:(ec + 1) * 512],
                                start=(dt == 0), stop=False)
                        nc.tensor.matmul(
                            ps,
                            ones_bf[0:1, 0:128],
                            bv_sb[0:1, ec * 512:(ec + 1) * 512],
                            start=False, stop=True)
                        nc.vector.tensor_copy(
                            out=vv[st][:, ec * 8:(ec + 1) * 8, 0:HD],
                            in_=ps.rearrange("p (h d) -> p h d", h=8))

            # ================= phase 2: attention + out-proj =================
            for t in range(NDT):
                nc.sync.dma_start(out=Wo_t[t], in_=WoT_d[t * 128:(t + 1) * 128, :])

            with tc.tile_pool(name="p2", bufs=1) as p2, \
                 tc.psum_pool(name="sp", bufs=2) as sp, \
                 tc.psum_pool(name="cp", bufs=2) as cp, \
                 tc.psum_pool(name="bp", bufs=1) as bp, \
                 tc.psum_pool(name="op", bufs=1) as op:

                def emit_norm(prev):
                    # normalization of the PREVIOUS head, deferred so its
                    # reciprocal latency hides under the current head's exps
                    ctx_prev, ht_p, hp_p, qc_p = prev
                    inv = p2.tile([1, 512], F32, name="inv", tag="inv",
                                  bufs=2)
                    nc.vector.reciprocal(inv, ctx_prev[64:65, :])
                    bc_ps = bp.tile([64, 512], F32, name="bc_ps", tag="bc",
                                    bufs=1)
                    nc.tensor.matmul(bc_ps, ones_r[0:1, 0:64], inv[0:1, :],
                                     start=True, stop=True)
                    bc_sb = p2.tile([64, 512], F32, name="bc_sb",
                                    tag="bc_sb", bufs=2)
                    nc.vector.tensor_copy(out=bc_sb, in_=bc_ps)
                    nc.vector.tensor_mul(
                        ctxn[ht_p][hp_p:hp_p + 64,
                                   qc_p * 512:(qc_p + 1) * 512],
                        ctx_prev[0:64, :], bc_sb)

                def emit_outproj(qc_o, et):
                    ps = op.tile([128, 512], F32, name="ops", tag="op",
                                 bufs=1)
                    for dt in range(NDT):
                        nc.tensor.matmul(
                            ps,
                            Wo_t[dt][:, et * 128:(et + 1) * 128],
                            ctxn[dt][:, qc_o * 512:(qc_o + 1) * 512],
                            start=(dt == 0), stop=(dt == NDT - 1))
                    osb = p2.tile([128, 512], F32, name="osb", tag="osb",
                                  bufs=2)
                    nc.vector.tensor_copy(out=osb, in_=ps)
                    nc.gpsimd.dma_start(
                        out=outT_d[et * 128:(et + 1) * 128,
                                   qc_o * 512:(qc_o + 1) * 512],
                        in_=osb)

                pending = None
                op_queue = []
                iters = [(qc, h, kh)
                         for qc in range(NQC)
                         for h in range(H)
                         for kh in range(NKT // 2)]

                def emit_sc(qc, h, kh):
                    ht, hp = h // 2, (h % 2) * 64
                    sc_ps = sp.tile([128, 1024], F32, name="sc_ps",
                                    tag="sc", bufs=2)
                    for j in range(2):
                        kt = kh * 2 + j
                        nc.tensor.matmul(
                            sc_ps[:, j * 512:(j + 1) * 512],
                            kT[ht][hp:hp + 64, kt * 128:(kt + 1) * 128],
                            qT[ht][hp:hp + 64, qc * 512:(qc + 1) * 512],
                            start=True, stop=True)
                    return sc_ps

                sc_next = emit_sc(*iters[0])
                ctx_ps = None
                for i, (qc, h, kh) in enumerate(iters):
                    sc_ps = sc_next
                    expT = p2.tile([128, 1024], BF16, name="expT",
                                   tag="expT", bufs=3)
                    nc.scalar.activation(
                        expT, sc_ps,
                        mybir.ActivationFunctionType.Exp,
                        scale=SCALE)
                    # next iteration's scores go ahead of this ctx so the
                    # PE keeps ScalarE fed across head boundaries
                    if i + 1 < len(iters):
                        sc_next = emit_sc(*iters[i + 1])
                    if kh == 0:
                        ctx_ps = cp.tile([65, 512], F32, name="ctx_ps",
                                         tag="ctx", bufs=2)
                    for j in range(2):
                        kt = kh * 2 + j
                        nc.tensor.matmul(
                            ctx_ps,
                            vv[kt][:, h, :],
                            expT[:, j * 512:(j + 1) * 512],
                            start=(kt == 0), stop=(kt == NKT - 1))
                    if kh == 3 and pending is not None:
                        emit_norm(pending)
                        pending = None
                    if kh == 6 and op_queue and h % 2 == 1:
                        emit_outproj(*op_queue.pop(0))
                    if kh == NKT // 2 - 1:
                        pending = (ctx_ps, h // 2, (h % 2) * 64, qc)
                        if qc == 0 and h == H - 1:
                            op_queue = [(0, et) for et in range(NET)]
                emit_norm(pending)
                for args in op_queue:
                    emit_outproj(*args)
                for et in range(NET):
                    emit_outproj(1, et)
    nc.compile()
    return nc


def _get_nc():
    global _NC_CACHE
    if _NC_CACHE is None:
        _NC_CACHE = build_nc()
    return _NC_CACHE


def _prep_maps(x, Wq, bq, Wk, bk, Wv, bv, Wo):
    bf = ml_dtypes.bfloat16
    WqT = np.ascontiguousarray(Wq.T).astype(bf)
    WkT = np.ascontiguousarray(Wk.T).astype(bf)
    WvT = np.ascontiguousarray(Wv.T).astype(bf)
    WoT = np.ascontiguousarray(Wo.T).astype(bf)
    bqt = np.ascontiguousarray(bq.reshape(NET, 128).T).astype(np.float32)
    bkt = np.ascontiguousarray(bk.reshape(NET, 128).T).astype(np.float32)
    bvr = np.ascontiguousarray(bv.reshape(1, D)).astype(bf)
    in_maps = []
    for c in range(8):
        b, hq = c // 2, c % 2
        xTb = np.ascontiguousarray(x[b].T).astype(bf)  # [D, S]
        if hq == 1:
            # rotate so local query half sits at columns [0, SQ)
            xTb = np.ascontiguousarray(
                np.concatenate([xTb[:, SQ:], xTb[:, :SQ]], axis=1))
        in_maps.append(dict(xT=xTb, WqT=WqT, WkT=WkT, WvT=WvT, WoT=WoT,
                            bqt=bqt, bkt=bkt, bvr=bvr))
    return in_maps


def run(x, Wq, bq, Wk, bk, Wv, bv, Wo, bo, trace=False, **spmd_kwargs):
    nc = _get_nc()
    in_maps = _prep_maps(x, Wq, bq, Wk, bk, Wv, bv, Wo)
    res = run_bass_kernel_spmd(nc, in_maps, core_ids=list(range(8)),
                               trace=trace, **spmd_kwargs)
    out = np.empty((B, S, D), np.float32)
    for c in range(8):
        b, hq = c // 2, c % 2
        out[b, hq * SQ:(hq + 1) * SQ, :] = np.asarray(
            res.results[c]["outT"], np.float32).T
    out += bo.astype(np.float32)
    return out, res


def kernel(x, Wq, bq, Wk, bk, Wv, bv, Wo, bo):
    out, _ = run(np.asarray(x, np.float32), np.asarray(Wq, np.float32),
                 np.asarray(bq, np.float32), np.asarray(Wk, np.float32),
                 np.asarray(bk, np.float32), np.asarray(Wv, np.float32),
                 np.asarray(bv, np.float32), np.asarray(Wo, np.float32),
                 np.asarray(bo, np.float32))
    return out



# revision 5
# speedup vs baseline: 1.3694x; 1.3694x over previous
"""Multi-head attention (B=4, S=2048, D=1024, H=16) on 8 Trainium2 cores.

Sharding: core c -> (batch b=c//2, query-half hq=c%2). Each core computes
K/V projections for its batch's full sequence (no collectives needed) and
attention + output projection for its 1024 query rows.

Device dataflow (all activations kept transposed, [feature, seq]):
  qT[e,s]   = WqT.T-contract  (lhsT=WqT[d,e] tiles, rhs=xT[d,s])
  kT[e,s]   = same with WkT
  v[s,e]    = lhsT=xT[d,s] tiles, rhs=WvT[d,e]  (+bias via K=1 ones matmul)
  per head, per 512-q chunk:
    scoresT[k,q] = kT_h.T-contract q  (K=64 matmuls, 4 k-tiles -> 4 psum banks)
    expT = ScalarE Exp(scale=0.125) over [128, 2048] psum -> bf16 sbuf
    ctxT[dv,q]  += [v_h | ones] @ expT   (row 64 = softmax denominator)
    normalize: reciprocal + PE broadcast outer-product + DVE multiply
  outT[e,q] = WoT.T-contract ctxnT  (bias bo added host-side)
Host: out[b, hq*1024:(hq+1)*1024, :] = outT.T + bo
"""

import numpy as np
import ml_dtypes

import concourse.bacc as bacc
import concourse.tile as tile
from concourse import mybir
from concourse.bass_utils import run_bass_kernel_spmd

B, S, D = 4, 2048, 1024
H, HD = 16, 64
SQ = 1024          # query rows per core
NDT = D // 128     # 8 d-tiles
NET = D // 128     # 8 e-tiles
NKT = S // 128     # 16 k-tiles
NST = S // 128     # 16 s-tiles
NQC = SQ // 512    # 2 q-chunks per core
BF16 = mybir.dt.bfloat16
F32 = mybir.dt.float32
SCALE = 1.0 / 8.0  # 1/sqrt(HD)

_NC_CACHE = None


def build_nc():
    nc = bacc.Bacc(None, target_bir_lowering=False, debug=True)

    xT_d = nc.declare_dram_parameter("xT", [D, S], BF16, isOutput=False)
    WqT_d = nc.declare_dram_parameter("WqT", [D, D], BF16, isOutput=False)
    WkT_d = nc.declare_dram_parameter("WkT", [D, D], BF16, isOutput=False)
    WvT_d = nc.declare_dram_parameter("WvT", [D, D], BF16, isOutput=False)
    WoT_d = nc.declare_dram_parameter("WoT", [D, D], BF16, isOutput=False)
    bqt_d = nc.declare_dram_parameter("bqt", [128, NET], F32, isOutput=False)
    bkt_d = nc.declare_dram_parameter("bkt", [128, NET], F32, isOutput=False)
    bvr_d = nc.declare_dram_parameter("bvr", [1, D], BF16, isOutput=False)
    outT_d = nc.declare_dram_parameter("outT", [D, SQ], F32, isOutput=True)

    with tile.TileContext(nc) as tc:
        with tc.tile_pool(name="resident", bufs=1) as res:
            # ---- resident SBUF tensors ----
            kT = [res.tile([128, S], BF16, name=f"kT{t}", tag=f"kT{t}")
                  for t in range(NET)]
            # per-head zero-padded q: head h's 64 dims sit at partitions
            # (h%2)*64, the other half is zero.  Scores matmuls can then
            # contract over the full 128 partitions (K=64 matmuls stream at
            # half rate on trn2; the zero rows make K=128 exact and fast).
            qTz = [res.tile([128, SQ], BF16, name=f"qTz{h}", tag=f"qTz{h}")
                   for h in range(H)]
            vv = [res.tile([128, H, HD + 1], BF16, name=f"v{t}", tag=f"v{t}")
                  for t in range(NST)]
            ctxn = [res.tile([128, SQ], BF16, name=f"ctxn{t}", tag=f"ctxn{t}")
                    for t in range(NDT)]
            Wo_t = [res.tile([128, D], BF16, name=f"Wo{t}", tag=f"Wo{t}")
                    for t in range(NDT)]
            bq_dma = res.tile([128, NET], F32, tag="bq_dma")
            bk_dma = res.tile([128, NET], F32, tag="bk_dma")
            bq_sb = res.tile([128, NET], F32, tag="bq_sb")
            bk_sb = res.tile([128, NET], F32, tag="bk_sb")
            bv_sb = res.tile([1, D], BF16, tag="bv_sb")
            ones_bf = res.tile([1, 128], BF16, tag="ones_bf")

            nc.sync.dma_start(out=bq_dma, in_=bqt_d[:, :])
            nc.sync.dma_start(out=bk_dma, in_=bkt_d[:, :])
            nc.sync.dma_start(out=bv_sb, in_=bvr_d[:, :])
            # TensorScalarPtr has a single sync-wait slot; route the biases
            # through DVE once so later readers rely on program order.
            nc.vector.tensor_copy(out=bq_sb, in_=bq_dma)
            nc.vector.tensor_copy(out=bk_sb, in_=bk_dma)
            nc.vector.memset(ones_bf, 1.0)
            for h in range(H):
                z0 = 64 if h % 2 == 0 else 0
                nc.vector.memset(qTz[h][z0:z0 + 64, :], 0.0)
            for t in range(NST):
                # only the denominator column; cols 0:HD are overwritten
                nc.vector.memset(vv[t][:, :, HD:HD + 1], 1.0)

            # ================= phase 1: projections =================
            with tc.tile_pool(name="p1", bufs=1) as p1, \
                 tc.psum_pool(name="pp", bufs=4) as pp:
                xT = [p1.tile([128, S], BF16, name=f"xT{t}", tag=f"xT{t}")
                      for t in range(NDT)]
                for t in range(NDT):
                    nc.sync.dma_start(out=xT[t], in_=xT_d[t * 128:(t + 1) * 128, :])

                # qT then kT: out[e_tile, s_chunk] accumulated over d
                for W_d, bias_sb, nsc, is_q in (
                    (WqT_d, bq_sb, NQC, True),   # q: only local 1024 cols
                    (WkT_d, bk_sb, S // 512, False),
                ):
                    w_t = []
                    for t in range(NDT):
                        wt = p1.tile([128, D], BF16, name=f"w{t}", tag="wrot",
                                     bufs=10)
                        nc.sync.dma_start(out=wt, in_=W_d[t * 128:(t + 1) * 128, :])
                        w_t.append(wt)
                    for et in range(NET):
                        for sc in range(nsc):
                            ps = pp.tile([128, 512], F32, name="ps", tag="proj")
                            for dt in range(NDT):
                                nc.tensor.matmul(
                                    ps,
                                    w_t[dt][:, et * 128:(et + 1) * 128],
                                    xT[dt][:, sc * 512: sc * 512 + 512],
                                    start=(dt == 0), stop=(dt == NDT - 1))
                            if is_q:
                                # split the two heads of this e-tile into
                                # their zero-padded per-head tiles
                                sl = slice(sc * 512, (sc + 1) * 512)
                                nc.vector.tensor_scalar_add(
                                    out=qTz[2 * et][0:64, sl],
                                    in0=ps[0:64, :],
                                    scalar1=bq_sb[0:64, et:et + 1])
                                nc.vector.tensor_scalar_add(
                                    out=qTz[2 * et + 1][64:128, sl],
                                    in0=ps[64:128, :],
                                    scalar1=bq_sb[64:128, et:et + 1])
                            else:
                                nc.vector.tensor_scalar_add(
                                    out=kT[et][:, sc * 512:(sc + 1) * 512],
                                    in0=ps,
                                    scalar1=bias_sb[:, et:et + 1])

                # v: out[s_tile, e_chunk] accumulated over d, + ones-row bias
                wv_t = []
                for t in range(NDT):
                    wt = p1.tile([128, D], BF16, name=f"wv{t}", tag="wrot",
                                 bufs=10)
                    nc.sync.dma_start(out=wt, in_=WvT_d[t * 128:(t + 1) * 128, :])
                    wv_t.append(wt)
                for st in range(NST):
                    for ec in range(D // 512):
                        ps = pp.tile([128, 512], F32, name="ps", tag="proj")
                        for dt in range(NDT):
                            nc.tensor.matmul(
                                ps,
                                xT[dt][:, st * 128:(st + 1) * 128],
                                wv_t[dt][:, ec * 512:(ec + 1) * 512],
                                start=(dt == 0), stop=False)
                        nc.tensor.matmul(
                            ps,
                            ones_bf[0:1, 0:128],
                            bv_sb[0:1, ec * 512:(ec + 1) * 512],
                            start=False, stop=True)
                        nc.vector.tensor_copy(
                            out=vv[st][:, ec * 8:(ec + 1) * 8, 0:HD],
                            in_=ps.rearrange("p (h d) -> p h d", h=8))

            # ================= phase 2: attention + out-proj =================
            for t in range(NDT):
                nc.sync.dma_start(out=Wo_t[t], in_=WoT_d[t * 128:(t + 1) * 128, :])

            with tc.tile_pool(name="p2", bufs=1) as p2, \
                 tc.psum_pool(name="sp", bufs=2) as sp, \
                 tc.psum_pool(name="cp", bufs=2) as cp, \
                 tc.psum_pool(name="op", bufs=1) as op:

                def emit_norm(prev):
                    # normalization of the PREVIOUS head, deferred so its
                    # broadcast latency hides under the current head's exps
                    ctx_prev, ht_p, hp_p, qc_p = prev
                    den = p2.tile([1, 512], F32, name="den", tag="den",
                                  bufs=2)
                    nc.vector.tensor_copy(out=den, in_=ctx_prev[64:65, :])
                    den_bc = p2.tile([64, 512], F32, name="den_bc",
                                     tag="den_bc", bufs=2)
                    nc.gpsimd.partition_broadcast(den_bc, den[0:1, :])
                    inv_bc = p2.tile([64, 512], F32, name="inv_bc",
                                     tag="inv_bc", bufs=2)
                    nc.vector.reciprocal(inv_bc, den_bc)
                    nc.vector.tensor_mul(
                        ctxn[ht_p][hp_p:hp_p + 64,
                                   qc_p * 512:(qc_p + 1) * 512],
                        ctx_prev[0:64, :], inv_bc)

                def emit_outproj(qc_o, et):
                    ps = op.tile([128, 512], F32, name="ops", tag="op",
                                 bufs=1)
                    for dt in range(NDT):
                        nc.tensor.matmul(
                            ps,
                            Wo_t[dt][:, et * 128:(et + 1) * 128],
                            ctxn[dt][:, qc_o * 512:(qc_o + 1) * 512],
                            start=(dt == 0), stop=(dt == NDT - 1))
                    osb = p2.tile([128, 512], F32, name="osb", tag="osb",
                                  bufs=2)
                    nc.vector.tensor_copy(out=osb, in_=ps)
                    nc.sync.dma_start(
                        out=outT_d[et * 128:(et + 1) * 128,
                                   qc_o * 512:(qc_o + 1) * 512],
                        in_=osb)

                pending = None
                op_queue = []
                iters = [(qc, h, kh)
                         for qc in range(NQC)
                         for h in range(H)
                         for kh in range(NKT // 2)]

                def emit_sc(qc, h, kh):
                    ht = h // 2
                    sc_ps = sp.tile([128, 1024], F32, name="sc_ps",
                                    tag="sc", bufs=2)
                    for j in range(2):
                        kt = kh * 2 + j
                        # K=128 contraction: the other head's partitions of
                        # qTz are zero, so only head h contributes.
                        nc.tensor.matmul(
                            sc_ps[:, j * 512:(j + 1) * 512],
                            kT[ht][:, kt * 128:(kt + 1) * 128],
                            qTz[h][:, qc * 512:(qc + 1) * 512],
                            start=True, stop=True)
                    return sc_ps

                sc_next = emit_sc(*iters[0])
                ctx_ps = None
                for i, (qc, h, kh) in enumerate(iters):
                    sc_ps = sc_next
                    expT = p2.tile([128, 1024], BF16, name="expT",
                                   tag="expT", bufs=3)
                    nc.scalar.activation(
                        expT, sc_ps,
                        mybir.ActivationFunctionType.Exp,
                        scale=SCALE)
                    # next iteration's scores go ahead of this ctx so the
                    # PE keeps ScalarE fed across head boundaries
                    if i + 1 < len(iters):
                        sc_next = emit_sc(*iters[i + 1])
                    if kh == 0:
                        ctx_ps = cp.tile([65, 512], F32, name="ctx_ps",
                                         tag="ctx", bufs=2)
                    for j in range(2):
                        kt = kh * 2 + j
                        nc.tensor.matmul(
                            ctx_ps,
                            vv[kt][:, h, :],
                            expT[:, j * 512:(j + 1) * 512],
                            start=(kt == 0), stop=(kt == NKT - 1))
                    if kh == 3 and pending is not None:
                        emit_norm(pending)
                        pending = None
                    if kh == 6 and op_queue and h % 2 == 1:
                        emit_outproj(*op_queue.pop(0))
                    if kh == NKT // 2 - 1:
                        pending = (ctx_ps, h // 2, (h % 2) * 64, qc)
                        if qc == 0 and h == H - 1:
                            op_queue = [(0, et) for et in range(NET)]
                emit_norm(pending)
                for args in op_queue:
                    emit_outproj(*args)
                for et in range(NET):
                    emit_outproj(1, et)
    nc.compile()
    return nc


def _get_nc():
    global _NC_CACHE
    if _NC_CACHE is None:
        _NC_CACHE = build_nc()
    return _NC_CACHE


def _prep_maps(x, Wq, bq, Wk, bk, Wv, bv, Wo):
    bf = ml_dtypes.bfloat16
    WqT = np.ascontiguousarray(Wq.T).astype(bf)
    WkT = np.ascontiguousarray(Wk.T).astype(bf)
    WvT = np.ascontiguousarray(Wv.T).astype(bf)
    WoT = np.ascontiguousarray(Wo.T).astype(bf)
    bqt = np.ascontiguousarray(bq.reshape(NET, 128).T).astype(np.float32)
    bkt = np.ascontiguousarray(bk.reshape(NET, 128).T).astype(np.float32)
    bvr = np.ascontiguousarray(bv.reshape(1, D)).astype(bf)
    in_maps = []
    for c in range(8):
        b, hq = c // 2, c % 2
        xTb = np.ascontiguousarray(x[b].T).astype(bf)  # [D, S]
        if hq == 1:
            # rotate so local query half sits at columns [0, SQ)
            xTb = np.ascontiguousarray(
                np.concatenate([xTb[:, SQ:], xTb[:, :SQ]], axis=1))
        in_maps.append(dict(xT=xTb, WqT=WqT, WkT=WkT, WvT=WvT, WoT=WoT,
                            bqt=bqt, bkt=bkt, bvr=bvr))
    return in_maps


def run(x, Wq, bq, Wk, bk, Wv, bv, Wo, bo, trace=False, **spmd_kwargs):
    nc = _get_nc()
    in_maps = _prep_maps(x, Wq, bq, Wk, bk, Wv, bv, Wo)
    res = run_bass_kernel_spmd(nc, in_maps, core_ids=list(range(8)),
                               trace=trace, **spmd_kwargs)
    out = np.empty((B, S, D), np.float32)
    for c in range(8):
        b, hq = c // 2, c % 2
        out[b, hq * SQ:(hq + 1) * SQ, :] = np.asarray(
            res.results[c]["outT"], np.float32).T
    out += bo.astype(np.float32)
    return out, res


def kernel(x, Wq, bq, Wk, bk, Wv, bv, Wo, bo):
    out, _ = run(np.asarray(x, np.float32), np.asarray(Wq, np.float32),
                 np.asarray(bq, np.float32), np.asarray(Wk, np.float32),
                 np.asarray(bk, np.float32), np.asarray(Wv, np.float32),
                 np.asarray(bv, np.float32), np.asarray(Wo, np.float32),
                 np.asarray(bo, np.float32))
    return out



# revision 8
# speedup vs baseline: 1.5148x; 1.1062x over previous
"""Multi-head attention (B=4, S=2048, D=1024, H=16) on 8 Trainium2 cores.

Sharding: core c -> (batch b=c//2, query-half hq=c%2). Each core computes
K/V projections for its batch's full sequence (no collectives needed) and
attention + output projection for its 1024 query rows.

Device dataflow (all activations kept transposed, [feature, seq]):
  qT[e,s]   = WqT.T-contract  (lhsT=WqT[d,e] tiles, rhs=xT[d,s])
  kT[e,s]   = same with WkT
  v[s,e]    = lhsT=xT[d,s] tiles, rhs=WvT[d,e]  (+bias via K=1 ones matmul)
  per head, per 512-q chunk:
    scoresT[k,q] = kT_h.T-contract q  (K=64 matmuls, 4 k-tiles -> 4 psum banks)
    expT = ScalarE Exp(scale=0.125) over [128, 2048] psum -> bf16 sbuf
    ctxT[dv,q]  += [v_h | ones] @ expT   (row 64 = softmax denominator)
    normalize: reciprocal + PE broadcast outer-product + DVE multiply
  outT[e,q] = WoT.T-contract ctxnT  (bias bo added host-side)
Host: out[b, hq*1024:(hq+1)*1024, :] = outT.T + bo
"""

import numpy as np
import ml_dtypes

import concourse.bacc as bacc
import concourse.tile as tile
from concourse import mybir
from concourse.bass_utils import run_bass_kernel_spmd

B, S, D = 4, 2048, 1024
H, HD = 16, 64
SQ = 1024          # query rows per core
NDT = D // 128     # 8 d-tiles
NET = D // 128     # 8 e-tiles
NKT = S // 128     # 16 k-tiles
NST = S // 128     # 16 s-tiles
NQC = SQ // 512    # 2 q-chunks per core
BF16 = mybir.dt.bfloat16
F32 = mybir.dt.float32
SCALE = 1.0 / 8.0  # 1/sqrt(HD)

_NC_CACHE = None


def build_nc():
    nc = bacc.Bacc(None, target_bir_lowering=False, debug=True)

    xT_d = nc.declare_dram_parameter("xT", [D, S], BF16, isOutput=False)
    WqT_d = nc.declare_dram_parameter("WqT", [D, D], BF16, isOutput=False)
    WkT_d = nc.declare_dram_parameter("WkT", [D, D], BF16, isOutput=False)
    WvT_d = nc.declare_dram_parameter("WvT", [D, D], BF16, isOutput=False)
    WoT_d = nc.declare_dram_parameter("WoT", [D, D], BF16, isOutput=False)
    bqt_d = nc.declare_dram_parameter("bqt", [128, NET], F32, isOutput=False)
    bkt_d = nc.declare_dram_parameter("bkt", [128, NET], F32, isOutput=False)
    bvr_d = nc.declare_dram_parameter("bvr", [1, D], BF16, isOutput=False)
    outT_d = nc.declare_dram_parameter("outT", [D, SQ], F32, isOutput=True)

    with tile.TileContext(nc) as tc:
        with tc.tile_pool(name="resident", bufs=1) as res:
            # ---- resident SBUF tensors ----
            kT = [res.tile([128, S], BF16, name=f"kT{t}", tag=f"kT{t}")
                  for t in range(NET)]
            # per-head zero-padded q: head h's 64 dims sit at partitions
            # (h%2)*64, the other half is zero.  Scores matmuls can then
            # contract over the full 128 partitions (K=64 matmuls stream at
            # half rate on trn2; the zero rows make K=128 exact and fast).
            qTz = [res.tile([128, SQ], BF16, name=f"qTz{h}", tag=f"qTz{h}")
                   for h in range(H)]
            vv = [res.tile([128, H, HD + 1], BF16, name=f"v{t}", tag=f"v{t}")
                  for t in range(NST)]
            # per-qc ctxn tiles so out-proj of one q-chunk doesn't pick up
            # false tile-level deps on the other chunk's normalizations
            ctxn = [[res.tile([128, 512], BF16, name=f"ctxn{qc}_{t}",
                              tag=f"ctxn{qc}_{t}") for t in range(NDT)]
                    for qc in range(NQC)]
            Wo_t = [res.tile([128, D], BF16, name=f"Wo{t}", tag=f"Wo{t}")
                    for t in range(NDT)]
            bq_dma = res.tile([128, NET], F32, tag="bq_dma")
            bk_dma = res.tile([128, NET], F32, tag="bk_dma")
            bq_sb = res.tile([128, NET], F32, tag="bq_sb")
            bk_sb = res.tile([128, NET], F32, tag="bk_sb")
            bv_sb = res.tile([1, D], BF16, tag="bv_sb")
            ones_bf = res.tile([1, 128], BF16, tag="ones_bf")

            nc.sync.dma_start(out=bq_dma, in_=bqt_d[:, :])
            nc.sync.dma_start(out=bk_dma, in_=bkt_d[:, :])
            nc.sync.dma_start(out=bv_sb, in_=bvr_d[:, :])
            # TensorScalarPtr has a single sync-wait slot; route the biases
            # through DVE once so later readers rely on program order.
            nc.vector.tensor_copy(out=bq_sb, in_=bq_dma)
            nc.vector.tensor_copy(out=bk_sb, in_=bk_dma)
            nc.vector.memset(ones_bf, 1.0)
            for h in range(H):
                z0 = 64 if h % 2 == 0 else 0
                nc.vector.memset(qTz[h][z0:z0 + 64, :], 0.0)
            for t in range(NST):
                # only the denominator column; cols 0:HD are overwritten
                nc.vector.memset(vv[t][:, :, HD:HD + 1], 1.0)

            # ================= phase 1: projections =================
            with tc.tile_pool(name="p1", bufs=1) as p1, \
                 tc.psum_pool(name="pp", bufs=4) as pp:
                xT = [p1.tile([128, S], BF16, name=f"xT{t}", tag=f"xT{t}")
                      for t in range(NDT)]
                for t in range(NDT):
                    nc.sync.dma_start(out=xT[t], in_=xT_d[t * 128:(t + 1) * 128, :])

                # qT then kT: out[e_tile, s_chunk] accumulated over d
                for W_d, bias_sb, nsc, is_q in (
                    (WqT_d, bq_sb, NQC, True),   # q: only local 1024 cols
                    (WkT_d, bk_sb, S // 512, False),
                ):
                    w_t = []
                    for t in range(NDT):
                        wt = p1.tile([128, D], BF16, name=f"w{t}", tag="wrot",
                                     bufs=10)
                        nc.sync.dma_start(out=wt, in_=W_d[t * 128:(t + 1) * 128, :])
                        w_t.append(wt)
                    for et in range(NET):
                        for sc in range(nsc):
                            ps = pp.tile([128, 512], F32, name="ps", tag="proj")
                            for dt in range(NDT):
                                nc.tensor.matmul(
                                    ps,
                                    w_t[dt][:, et * 128:(et + 1) * 128],
                                    xT[dt][:, sc * 512: sc * 512 + 512],
                                    start=(dt == 0), stop=(dt == NDT - 1))
                            if is_q:
                                # split the two heads of this e-tile into
                                # their zero-padded per-head tiles
                                sl = slice(sc * 512, (sc + 1) * 512)
                                nc.vector.tensor_scalar_add(
                                    out=qTz[2 * et][0:64, sl],
                                    in0=ps[0:64, :],
                                    scalar1=bq_sb[0:64, et:et + 1])
                                nc.vector.tensor_scalar_add(
                                    out=qTz[2 * et + 1][64:128, sl],
                                    in0=ps[64:128, :],
                                    scalar1=bq_sb[64:128, et:et + 1])
                            else:
                                nc.vector.tensor_scalar_add(
                                    out=kT[et][:, sc * 512:(sc + 1) * 512],
                                    in0=ps,
                                    scalar1=bias_sb[:, et:et + 1])

                # v: out[s_tile, e_chunk] accumulated over d, + ones-row bias
                wv_t = []
                for t in range(NDT):
                    wt = p1.tile([128, D], BF16, name=f"wv{t}", tag="wrot",
                                 bufs=10)
                    nc.sync.dma_start(out=wt, in_=WvT_d[t * 128:(t + 1) * 128, :])
                    wv_t.append(wt)
                for st in range(NST):
                    for ec in range(D // 512):
                        ps = pp.tile([128, 512], F32, name="ps", tag="proj")
                        for dt in range(NDT):
                            nc.tensor.matmul(
                                ps,
                                xT[dt][:, st * 128:(st + 1) * 128],
                                wv_t[dt][:, ec * 512:(ec + 1) * 512],
                                start=(dt == 0), stop=False)
                        nc.tensor.matmul(
                            ps,
                            ones_bf[0:1, 0:128],
                            bv_sb[0:1, ec * 512:(ec + 1) * 512],
                            start=False, stop=True)
                        nc.vector.tensor_copy(
                            out=vv[st][:, ec * 8:(ec + 1) * 8, 0:HD],
                            in_=ps.rearrange("p (h d) -> p h d", h=8))

            # ================= phase 2: attention + out-proj =================
            for t in range(NDT):
                nc.sync.dma_start(out=Wo_t[t], in_=WoT_d[t * 128:(t + 1) * 128, :])

            with tc.tile_pool(name="p2", bufs=1) as p2, \
                 tc.psum_pool(name="sp", bufs=2) as sp, \
                 tc.psum_pool(name="cp", bufs=3) as cp, \
                 tc.psum_pool(name="op", bufs=1) as op:

                def emit_norm(prev):
                    # normalization of the PREVIOUS head, deferred so its
                    # broadcast latency hides under the current head's exps
                    ctx_prev, ht_p, hp_p, qc_p = prev
                    den = p2.tile([1, 512], F32, name="den", tag="den",
                                  bufs=2)
                    nc.vector.tensor_copy(out=den, in_=ctx_prev[64:65, :])
                    den_bc = p2.tile([64, 512], F32, name="den_bc",
                                     tag="den_bc", bufs=2)
                    nc.gpsimd.partition_broadcast(den_bc, den[0:1, :])
                    inv_bc = p2.tile([64, 512], F32, name="inv_bc",
                                     tag="inv_bc", bufs=2)
                    # denominators are sums of exps (>0, moderate range):
                    # the fast approx (~18 bits) is far below the rel-err
                    # budget and 5x cheaper than the exact table op
                    nc.vector.reciprocal_approx_fast(inv_bc, den_bc)
                    nc.vector.tensor_mul(
                        ctxn[qc_p][ht_p][hp_p:hp_p + 64, :],
                        ctx_prev[0:64, :], inv_bc)

                def emit_outproj(qc_o, et):
                    ps = op.tile([128, 512], F32, name="ops", tag="op",
                                 bufs=1)
                    for dt in range(NDT):
                        nc.tensor.matmul(
                            ps,
                            Wo_t[dt][:, et * 128:(et + 1) * 128],
                            ctxn[qc_o][dt][:, :],
                            start=(dt == 0), stop=(dt == NDT - 1))
                    osb = p2.tile([128, 512], F32, name="osb", tag="osb",
                                  bufs=2)
                    nc.vector.tensor_copy(out=osb, in_=ps)
                    nc.sync.dma_start(
                        out=outT_d[et * 128:(et + 1) * 128,
                                   qc_o * 512:(qc_o + 1) * 512],
                        in_=osb)

                pending = None
                op_queue = []
                iters = [(qc, h, kh)
                         for qc in range(NQC)
                         for h in range(H)
                         for kh in range(NKT // 2)]

                def emit_sc(qc, h, kh):
                    ht = h // 2
                    sc_ps = sp.tile([128, 1024], F32, name="sc_ps",
                                    tag="sc", bufs=2)
                    for j in range(2):
                        kt = kh * 2 + j
                        # K=128 contraction: the other head's partitions of
                        # qTz are zero, so only head h contributes.
                        nc.tensor.matmul(
                            sc_ps[:, j * 512:(j + 1) * 512],
                            kT[ht][:, kt * 128:(kt + 1) * 128],
                            qTz[h][:, qc * 512:(qc + 1) * 512],
                            start=True, stop=True)
                    return sc_ps

                sc_next = emit_sc(*iters[0])
                ctx_ps = None
                for i, (qc, h, kh) in enumerate(iters):
                    sc_ps = sc_next
                    expT = p2.tile([128, 1024], BF16, name="expT",
                                   tag="expT", bufs=3)
                    nc.scalar.activation(
                        expT, sc_ps,
                        mybir.ActivationFunctionType.Exp,
                        scale=SCALE)
                    # next iteration's scores go ahead of this ctx so the
                    # PE keeps ScalarE fed across head boundaries
                    if i + 1 < len(iters):
                        sc_next = emit_sc(*iters[i + 1])
                    if kh == 0:
                        ctx_ps = cp.tile([65, 512], F32, name="ctx_ps",
                                         tag="ctx", bufs=2)
                    for j in range(2):
                        kt = kh * 2 + j
                        nc.tensor.matmul(
                            ctx_ps,
                            vv[kt][:, h, :],
                            expT[:, j * 512:(j + 1) * 512],
                            start=(kt == 0), stop=(kt == NKT - 1))
                    if kh == 1 and pending is not None:
                        emit_norm(pending)
                        pending = None
                    if kh == 6 and op_queue and h % 2 == 1:
                        emit_outproj(*op_queue.pop(0))
                    if kh == NKT // 2 - 1:
                        pending = (ctx_ps, h // 2, (h % 2) * 64, qc)
                        if qc == 0 and h == H - 1:
                            op_queue = [(0, et) for et in range(NET)]
                emit_norm(pending)
                for args in op_queue:
                    emit_outproj(*args)
                for et in range(NET):
                    emit_outproj(1, et)
    nc.compile()
    return nc


def _get_nc():
    global _NC_CACHE
    if _NC_CACHE is None:
        _NC_CACHE = build_nc()
    return _NC_CACHE


def _prep_maps(x, Wq, bq, Wk, bk, Wv, bv, Wo):
    bf = ml_dtypes.bfloat16
    WqT = np.ascontiguousarray(Wq.T).astype(bf)
    WkT = np.ascontiguousarray(Wk.T).astype(bf)
    WvT = np.ascontiguousarray(Wv.T).astype(bf)
    WoT = np.ascontiguousarray(Wo.T).astype(bf)
    bqt = np.ascontiguousarray(bq.reshape(NET, 128).T).astype(np.float32)
    bkt = np.ascontiguousarray(bk.reshape(NET, 128).T).astype(np.float32)
    bvr = np.ascontiguousarray(bv.reshape(1, D)).astype(bf)
    in_maps = []
    for c in range(8):
        b, hq = c // 2, c % 2
        xTb = np.ascontiguousarray(x[b].T).astype(bf)  # [D, S]
        if hq == 1:
            # rotate so local query half sits at columns [0, SQ)
            xTb = np.ascontiguousarray(
                np.concatenate([xTb[:, SQ:], xTb[:, :SQ]], axis=1))
        in_maps.append(dict(xT=xTb, WqT=WqT, WkT=WkT, WvT=WvT, WoT=WoT,
                            bqt=bqt, bkt=bkt, bvr=bvr))
    return in_maps


def run(x, Wq, bq, Wk, bk, Wv, bv, Wo, bo, trace=False, **spmd_kwargs):
    nc = _get_nc()
    in_maps = _prep_maps(x, Wq, bq, Wk, bk, Wv, bv, Wo)
    res = run_bass_kernel_spmd(nc, in_maps, core_ids=list(range(8)),
                               trace=trace, **spmd_kwargs)
    out = np.empty((B, S, D), np.float32)
    for c in range(8):
        b, hq = c // 2, c % 2
        out[b, hq * SQ:(hq + 1) * SQ, :] = np.asarray(
            res.results[c]["outT"], np.float32).T
    out += bo.astype(np.float32)
    return out, res


def kernel(x, Wq, bq, Wk, bk, Wv, bv, Wo, bo):
    out, _ = run(np.asarray(x, np.float32), np.asarray(Wq, np.float32),
                 np.asarray(bq, np.float32), np.asarray(Wk, np.float32),
                 np.asarray(bk, np.float32), np.asarray(Wv, np.float32),
                 np.asarray(bv, np.float32), np.asarray(Wo, np.float32),
                 np.asarray(bo, np.float32))
    return out



# revision 9
# speedup vs baseline: 1.5782x; 1.0419x over previous
"""Multi-head attention (B=4, S=2048, D=1024, H=16) on 8 Trainium2 cores.

Sharding: core c -> (batch b=c//2, query-half hq=c%2). Each core computes
K/V projections for its batch's full sequence (no collectives needed) and
attention + output projection for its 1024 query rows.

Device dataflow (all activations kept transposed, [feature, seq]):
  qT[e,s]   = WqT.T-contract  (lhsT=WqT[d,e] tiles, rhs=xT[d,s])
  kT[e,s]   = same with WkT
  v[s,e]    = lhsT=xT[d,s] tiles, rhs=WvT[d,e]  (+bias via K=1 ones matmul)
  per head, per 512-q chunk:
    scoresT[k,q] = kT_h.T-contract q  (K=64 matmuls, 4 k-tiles -> 4 psum banks)
    expT = ScalarE Exp(scale=0.125) over [128, 2048] psum -> bf16 sbuf
    ctxT[dv,q]  += [v_h | ones] @ expT   (row 64 = softmax denominator)
    normalize: reciprocal + PE broadcast outer-product + DVE multiply
  outT[e,q] = WoT.T-contract ctxnT  (bias bo added host-side)
Host: out[b, hq*1024:(hq+1)*1024, :] = outT.T + bo
"""

import numpy as np
import ml_dtypes

import concourse.bacc as bacc
import concourse.tile as tile
from concourse import mybir
from concourse.bass_utils import run_bass_kernel_spmd

B, S, D = 4, 2048, 1024
H, HD = 16, 64
SQ = 1024          # query rows per core
NDT = D // 128     # 8 d-tiles
NET = D // 128     # 8 e-tiles
NKT = S // 128     # 16 k-tiles
NST = S // 128     # 16 s-tiles
NQC = SQ // 512    # 2 q-chunks per core
BF16 = mybir.dt.bfloat16
F32 = mybir.dt.float32
SCALE = 1.0 / 8.0  # 1/sqrt(HD)

_NC_CACHE = None


def build_nc():
    nc = bacc.Bacc(None, target_bir_lowering=False, debug=True)

    xT_d = nc.declare_dram_parameter("xT", [D, S], BF16, isOutput=False)
    WqT_d = nc.declare_dram_parameter("WqT", [D, D], BF16, isOutput=False)
    WkT_d = nc.declare_dram_parameter("WkT", [D, D], BF16, isOutput=False)
    WvT_d = nc.declare_dram_parameter("WvT", [D, D], BF16, isOutput=False)
    WoT_d = nc.declare_dram_parameter("WoT", [D, D], BF16, isOutput=False)
    bqt_d = nc.declare_dram_parameter("bqt", [128, NET], F32, isOutput=False)
    bkt_d = nc.declare_dram_parameter("bkt", [128, NET], F32, isOutput=False)
    bvr_d = nc.declare_dram_parameter("bvr", [1, D], BF16, isOutput=False)
    outT_d = nc.declare_dram_parameter("outT", [D, SQ], F32, isOutput=True)

    with tile.TileContext(nc) as tc:
        with tc.tile_pool(name="resident", bufs=1) as res:
            # ---- resident SBUF tensors ----
            kT = [res.tile([128, S], BF16, name=f"kT{t}", tag=f"kT{t}")
                  for t in range(NET)]
            # per-head zero-padded q: head h's 64 dims sit at partitions
            # (h%2)*64, the other half is zero.  Scores matmuls can then
            # contract over the full 128 partitions (K=64 matmuls stream at
            # half rate on trn2; the zero rows make K=128 exact and fast).
            qTz = [res.tile([128, SQ], BF16, name=f"qTz{h}", tag=f"qTz{h}")
                   for h in range(H)]
            vv = [res.tile([128, H, HD + 1], BF16, name=f"v{t}", tag=f"v{t}")
                  for t in range(NST)]
            # per-qc ctxn tiles so out-proj of one q-chunk doesn't pick up
            # false tile-level deps on the other chunk's normalizations
            ctxn = [[res.tile([128, 512], BF16, name=f"ctxn{qc}_{t}",
                              tag=f"ctxn{qc}_{t}") for t in range(NDT)]
                    for qc in range(NQC)]
            Wo_t = [res.tile([128, D], BF16, name=f"Wo{t}", tag=f"Wo{t}")
                    for t in range(NDT)]
            bq_dma = res.tile([128, NET], F32, tag="bq_dma")
            bk_dma = res.tile([128, NET], F32, tag="bk_dma")
            bq_sb = res.tile([128, NET], F32, tag="bq_sb")
            bk_sb = res.tile([128, NET], F32, tag="bk_sb")
            bv_sb = res.tile([1, D], BF16, tag="bv_sb")
            ones_bf = res.tile([1, 128], BF16, tag="ones_bf")

            nc.sync.dma_start(out=bq_dma, in_=bqt_d[:, :])
            nc.sync.dma_start(out=bk_dma, in_=bkt_d[:, :])
            nc.sync.dma_start(out=bv_sb, in_=bvr_d[:, :])
            # TensorScalarPtr has a single sync-wait slot; route the biases
            # through DVE once so later readers rely on program order.
            nc.vector.tensor_copy(out=bq_sb, in_=bq_dma)
            nc.vector.tensor_copy(out=bk_sb, in_=bk_dma)
            nc.vector.memset(ones_bf, 1.0)
            for h in range(H):
                z0 = 64 if h % 2 == 0 else 0
                nc.vector.memset(qTz[h][z0:z0 + 64, :], 0.0)
            for t in range(NST):
                # only the denominator column; cols 0:HD are overwritten
                nc.vector.memset(vv[t][:, :, HD:HD + 1], 1.0)

            # ================= phase 1: projections =================
            with tc.tile_pool(name="p1", bufs=1) as p1, \
                 tc.psum_pool(name="pp", bufs=4) as pp:
                xT = [p1.tile([128, S], BF16, name=f"xT{t}", tag=f"xT{t}")
                      for t in range(NDT)]
                for t in range(NDT):
                    nc.sync.dma_start(out=xT[t], in_=xT_d[t * 128:(t + 1) * 128, :])

                # qT then kT: out[e_tile, s_chunk] accumulated over d
                for W_d, bias_sb, nsc, is_q in (
                    (WqT_d, bq_sb, NQC, True),   # q: only local 1024 cols
                    (WkT_d, bk_sb, S // 512, False),
                ):
                    w_t = []
                    for t in range(NDT):
                        wt = p1.tile([128, D], BF16, name=f"w{t}", tag="wrot",
                                     bufs=10)
                        nc.sync.dma_start(out=wt, in_=W_d[t * 128:(t + 1) * 128, :])
                        w_t.append(wt)
                    for et in range(NET):
                        for sc in range(nsc):
                            ps = pp.tile([128, 512], F32, name="ps", tag="proj")
                            for dt in range(NDT):
                                nc.tensor.matmul(
                                    ps,
                                    w_t[dt][:, et * 128:(et + 1) * 128],
                                    xT[dt][:, sc * 512: sc * 512 + 512],
                                    start=(dt == 0), stop=(dt == NDT - 1))
                            if is_q:
                                # split the two heads of this e-tile into
                                # their zero-padded per-head tiles
                                sl = slice(sc * 512, (sc + 1) * 512)
                                nc.vector.tensor_scalar_add(
                                    out=qTz[2 * et][0:64, sl],
                                    in0=ps[0:64, :],
                                    scalar1=bq_sb[0:64, et:et + 1])
                                nc.vector.tensor_scalar_add(
                                    out=qTz[2 * et + 1][64:128, sl],
                                    in0=ps[64:128, :],
                                    scalar1=bq_sb[64:128, et:et + 1])
                            else:
                                nc.vector.tensor_scalar_add(
                                    out=kT[et][:, sc * 512:(sc + 1) * 512],
                                    in0=ps,
                                    scalar1=bias_sb[:, et:et + 1])

                # v: out[s_tile, e_chunk] accumulated over d, + ones-row bias
                wv_t = []
                for t in range(NDT):
                    wt = p1.tile([128, D], BF16, name=f"wv{t}", tag="wrot",
                                 bufs=10)
                    nc.sync.dma_start(out=wt, in_=WvT_d[t * 128:(t + 1) * 128, :])
                    wv_t.append(wt)
                for st in range(NST):
                    for ec in range(D // 512):
                        ps = pp.tile([128, 512], F32, name="ps", tag="proj")
                        for dt in range(NDT):
                            nc.tensor.matmul(
                                ps,
                                xT[dt][:, st * 128:(st + 1) * 128],
                                wv_t[dt][:, ec * 512:(ec + 1) * 512],
                                start=(dt == 0), stop=False)
                        nc.tensor.matmul(
                            ps,
                            ones_bf[0:1, 0:128],
                            bv_sb[0:1, ec * 512:(ec + 1) * 512],
                            start=False, stop=True)
                        nc.vector.tensor_copy(
                            out=vv[st][:, ec * 8:(ec + 1) * 8, 0:HD],
                            in_=ps.rearrange("p (h d) -> p h d", h=8))

            # ================= phase 2: attention + out-proj =================
            for t in range(NDT):
                nc.sync.dma_start(out=Wo_t[t], in_=WoT_d[t * 128:(t + 1) * 128, :])

            with tc.tile_pool(name="p2", bufs=1) as p2, \
                 tc.psum_pool(name="sp", bufs=2) as sp, \
                 tc.psum_pool(name="cp", bufs=3) as cp, \
                 tc.psum_pool(name="op", bufs=1) as op:

                def emit_norm(prev):
                    # normalization of the PREVIOUS head, deferred so its
                    # broadcast latency hides under the current head's exps
                    ctx_prev, ht_p, hp_p, qc_p = prev
                    den = p2.tile([1, 512], F32, name="den", tag="den",
                                  bufs=2)
                    nc.vector.tensor_copy(out=den, in_=ctx_prev[64:65, :])
                    den_bc = p2.tile([64, 512], F32, name="den_bc",
                                     tag="den_bc", bufs=2)
                    nc.gpsimd.partition_broadcast(den_bc, den[0:1, :])
                    inv_bc = p2.tile([64, 512], F32, name="inv_bc",
                                     tag="inv_bc", bufs=2)
                    # denominators are sums of exps (>0, moderate range):
                    # the fast approx (~18 bits) is far below the rel-err
                    # budget and 5x cheaper than the exact table op
                    nc.vector.reciprocal_approx_fast(inv_bc, den_bc)
                    nc.vector.tensor_mul(
                        ctxn[qc_p][ht_p][hp_p:hp_p + 64, :],
                        ctx_prev[0:64, :], inv_bc)

                def emit_outproj(qc_o, et):
                    ps = op.tile([128, 512], F32, name="ops", tag="op",
                                 bufs=1)
                    for dt in range(NDT):
                        nc.tensor.matmul(
                            ps,
                            Wo_t[dt][:, et * 128:(et + 1) * 128],
                            ctxn[qc_o][dt][:, :],
                            start=(dt == 0), stop=(dt == NDT - 1))
                    osb = p2.tile([128, 512], F32, name="osb", tag="osb",
                                  bufs=2)
                    nc.vector.tensor_copy(out=osb, in_=ps)
                    nc.sync.dma_start(
                        out=outT_d[et * 128:(et + 1) * 128,
                                   qc_o * 512:(qc_o + 1) * 512],
                        in_=osb)

                pending = None
                op_queue = []
                iters = [(qc, h, kh)
                         for qc in range(NQC)
                         for h in range(H)
                         for kh in range(NKT // 2)]

                def emit_sc(qc, h, kh):
                    ht = h // 2
                    sc_ps = sp.tile([128, 1024], F32, name="sc_ps",
                                    tag="sc", bufs=2)
                    for j in range(2):
                        kt = kh * 2 + j
                        # K=128 contraction: the other head's partitions of
                        # qTz are zero, so only head h contributes.
                        nc.tensor.matmul(
                            sc_ps[:, j * 512:(j + 1) * 512],
                            kT[ht][:, kt * 128:(kt + 1) * 128],
                            qTz[h][:, qc * 512:(qc + 1) * 512],
                            start=True, stop=True)
                    return sc_ps

                ctx_state = {"ps": None}

                def emit_ctx_step(expT, qc, h, kh):
                    # ctx runs one iteration behind exp so the PE never
                    # waits on a just-signaled ScalarE semaphore
                    nonlocal pending, op_queue
                    if kh == 0:
                        ctx_state["ps"] = cp.tile([65, 512], F32,
                                                  name="ctx_ps", tag="ctx",
                                                  bufs=3)
                    ctx_ps = ctx_state["ps"]
                    for j in range(2):
                        kt = kh * 2 + j
                        nc.tensor.matmul(
                            ctx_ps,
                            vv[kt][:, h, :],
                            expT[:, j * 512:(j + 1) * 512],
                            start=(kt == 0), stop=(kt == NKT - 1))
                    if kh == NKT // 2 - 1:
                        pending = (ctx_ps, h // 2, (h % 2) * 64, qc)
                        if qc == 0 and h == H - 1:
                            op_queue = [(0, et) for et in range(NET)]

                sc_next = emit_sc(*iters[0])
                delayed = []
                for i, (qc, h, kh) in enumerate(iters):
                    sc_ps = sc_next
                    expT = p2.tile([128, 1024], BF16, name="expT",
                                   tag="expT", bufs=4)
                    nc.scalar.activation(
                        expT, sc_ps,
                        mybir.ActivationFunctionType.Exp,
                        scale=SCALE)
                    # next iteration's scores go ahead of this ctx so the
                    # PE keeps ScalarE fed across head boundaries
                    if i + 1 < len(iters):
                        sc_next = emit_sc(*iters[i + 1])
                    delayed.append((expT, qc, h, kh))
                    if len(delayed) > 1:
                        emit_ctx_step(*delayed.pop(0))
                    if kh == 2 and pending is not None:
                        emit_norm(pending)
                        pending = None
                    if kh == 6 and op_queue and h % 2 == 1:
                        emit_outproj(*op_queue.pop(0))
                while delayed:
                    emit_ctx_step(*delayed.pop(0))
                emit_norm(pending)
                for args in op_queue:
                    emit_outproj(*args)
                for et in range(NET):
                    emit_outproj(1, et)
    nc.compile()
    return nc


def _get_nc():
    global _NC_CACHE
    if _NC_CACHE is None:
        _NC_CACHE = build_nc()
    return _NC_CACHE


def _prep_maps(x, Wq, bq, Wk, bk, Wv, bv, Wo):
    bf = ml_dtypes.bfloat16
    WqT = np.ascontiguousarray(Wq.T).astype(bf)
    WkT = np.ascontiguousarray(Wk.T).astype(bf)
    WvT = np.ascontiguousarray(Wv.T).astype(bf)
    WoT = np.ascontiguousarray(Wo.T).astype(bf)
    bqt = np.ascontiguousarray(bq.reshape(NET, 128).T).astype(np.float32)
    bkt = np.ascontiguousarray(bk.reshape(NET, 128).T).astype(np.float32)
    bvr = np.ascontiguousarray(bv.reshape(1, D)).astype(bf)
    in_maps = []
    for c in range(8):
        b, hq = c // 2, c % 2
        xTb = np.ascontiguousarray(x[b].T).astype(bf)  # [D, S]
        if hq == 1:
            # rotate so local query half sits at columns [0, SQ)
            xTb = np.ascontiguousarray(
                np.concatenate([xTb[:, SQ:], xTb[:, :SQ]], axis=1))
        in_maps.append(dict(xT=xTb, WqT=WqT, WkT=WkT, WvT=WvT, WoT=WoT,
                            bqt=bqt, bkt=bkt, bvr=bvr))
    return in_maps


def run(x, Wq, bq, Wk, bk, Wv, bv, Wo, bo, trace=False, **spmd_kwargs):
    nc = _get_nc()
    in_maps = _prep_maps(x, Wq, bq, Wk, bk, Wv, bv, Wo)
    res = run_bass_kernel_spmd(nc, in_maps, core_ids=list(range(8)),
                               trace=trace, **spmd_kwargs)
    out = np.empty((B, S, D), np.float32)
    for c in range(8):
        b, hq = c // 2, c % 2
        out[b, hq * SQ:(hq + 1) * SQ, :] = np.asarray(
            res.results[c]["outT"], np.float32).T
    out += bo.astype(np.float32)
    return out, res


def kernel(x, Wq, bq, Wk, bk, Wv, bv, Wo, bo):
    out, _ = run(np.asarray(x, np.float32), np.asarray(Wq, np.float32),
                 np.asarray(bq, np.float32), np.asarray(Wk, np.float32),
                 np.asarray(bk, np.float32), np.asarray(Wv, np.float32),
                 np.asarray(bv, np.float32), np.asarray(Wo, np.float32),
                 np.asarray(bo, np.float32))
    return out

